# revision 32
# baseline (speedup 1.0000x reference)
"""Trainium2 Bass kernel for Transformer-XL style relative-position MHSA.

Problem: nn_MultiHeadSelfAttention_14989435863450
  B=2, S=2048, D=512, H=8, dh=64, fp32 I/O.

Sharding (8 cores): core c -> batch b = c//4, head pair h0 = 2*(c%4).
Each core computes its 2 heads' attention and the partial output
projection (out_slice @ Wo[slice]); host sums 4 partials per batch and
adds the constant (bv @ Wo + bo) row vector.

Math folds (exact):
  - bq folds into u,v:  u_eff = 64*(u + bq)/sqrt(D)  (64x score scaling
    for fp8 range; exp() applies scale=1/64)
  - bk adds a per-query-row constant to scores -> cancels in softmax
  - bv contributes attn-weighted 1 * bv = bv -> host-side constant
  - softmax normalization folded into the final Wo-projection evac
    (per-head reciprocal row-sum as scale)

Relative shift: pos scores are streamed to a DRAM buffer PB[S, S+1]
(fp8e4m3, 64x scaled) with rows [0 | posrow_i]; reading
PB.flat[S : S + S*S] as [S, S] is exactly Transformer-XL's
pad-reshape-slice shift (including the wrap).

v3 pipeline (from NTFF profiling of v1/v2):
  - all matmul inputs bf16; pos scores fp8 through DRAM
  - pos runs TWO q-blocks ahead; the shifted read for block ib is
    issued one iteration early (it needs PB rows through block ib+1)
  - the shifted pos tile is added into the content PSUM by an
    identity-weight matmul on the PE; exp() reads PSUM directly
    (no DVE add pass, no sc tile)
  - attention transposes via DMA XBAR on the SP ring; attn@v as
    column passes over the assembled atT
  - elementwise: DVE does pos-score fp8 evacs, ACT does exp
"""

import math
from contextlib import ExitStack

import numpy as np
import ml_dtypes

import concourse.bass as bass
import concourse.bacc as bacc_mod
import concourse.mybir as mybir
import concourse.tile as tile
from concourse.bass import ts, ds
from concourse.bass_utils import run_bass_kernel_spmd
from concourse.masks import make_identity

FP32 = mybir.dt.float32
BF16 = mybir.dt.bfloat16
FP8 = mybir.dt.float8e4

D_MODEL = 512
NUM_HEADS = 8
D_HEAD = 64
DH2 = 2 * D_HEAD
B_FULL = 2
S_FULL = 2048
P = 128
CH = 512                      # PSUM bank chunk (fp32)
ISQ = 1.0 / math.sqrt(D_MODEL)
SCALE = 64.0                  # fp8 range scaling for pos scores

Exp = mybir.ActivationFunctionType.Exp
ADD = mybir.AluOpType.add
MULT = mybir.AluOpType.mult

BF16NP = ml_dtypes.bfloat16


def build_nc(S=S_FULL):
    """Build the single-core Bass program (SPMD: same program, 8 cores)."""
    nc = bacc_mod.Bacc()
    NB = S // P               # q blocks
    NK = S // P               # key tiles
    KD = D_MODEL // P         # contraction tiles over D
    NCH = S // CH             # 512-chunks per row
    HB = NB // 2              # q blocks per half
    HS = S // 2               # columns per half

    xT = nc.declare_dram_parameter("xT", [D_MODEL, S], BF16, isOutput=False)
    posT = nc.declare_dram_parameter("posT", [D_MODEL, S], BF16, isOutput=False)
    Wq = nc.declare_dram_parameter("Wq", [D_MODEL, DH2], BF16, isOutput=False)
    Wk = nc.declare_dram_parameter("Wk", [D_MODEL, DH2], BF16, isOutput=False)
    Wv = nc.declare_dram_parameter("Wv", [D_MODEL, DH2], BF16, isOutput=False)
    Wp = nc.declare_dram_parameter("Wp", [D_MODEL, DH2], BF16, isOutput=False)
    Wo = nc.declare_dram_parameter("Wo", [DH2, D_MODEL], BF16, isOutput=False)
    ueff = nc.declare_dram_parameter("ueff", [DH2, 1], FP32, isOutput=False)
    veff = nc.declare_dram_parameter("veff", [DH2, 1], FP32, isOutput=False)
    out_partial = nc.declare_dram_parameter("out_partial", [S, D_MODEL], FP32, isOutput=True)

    with ExitStack() as ctx:
        tc = ctx.enter_context(tile.TileContext(nc))
        consts = ctx.enter_context(tc.tile_pool(name="consts", bufs=1))
        dram = ctx.enter_context(tc.tile_pool(name="dram", bufs=1, space="DRAM"))

        # ---- persistent SBUF ----
        qTu = consts.tile([DH2, S], BF16)
        qTv = consts.tile([DH2, S], BF16)
        kT = consts.tile([DH2, S], BF16)
        pT = consts.tile([DH2, S], BF16)
        vv = consts.tile([P, NK, DH2], BF16)      # natural [key, dh2]
        Wo_sb = consts.tile([DH2, D_MODEL], BF16)
        ueff_sb = consts.tile([DH2, 1], FP32)
        veff_sb = consts.tile([DH2, 1], FP32)
        ident = consts.tile([P, P], FP8)
        recs = [consts.tile([P, NB], FP32, name=f"rec{h}") for h in range(2)]
        o_both = consts.tile([DH2, S], BF16)      # unnormalized attn@v, [dh2, q]
        # vv with a ones column appended per head: attn@v's 65th output row
        # is then the softmax row-sum (no activation accumulator needed)
        vv_aug = [consts.tile([P, NK, D_HEAD + 1], BF16, name=f"vva{h}") for h in range(2)]
        sums_free = [consts.tile([1, S], FP32, name=f"sums{h}") for h in range(2)]

        make_identity(nc, ident[:])
        nc.scalar.dma_start(Wo_sb[:], Wo[:, :])
        nc.scalar.dma_start(ueff_sb[:], ueff[:, :])
        nc.scalar.dma_start(veff_sb[:], veff[:, :])

        # ---- load + projections (scoped: frees SBUF/PSUM after) ----
        with tc.tile_pool(name="loadp", bufs=1) as loadp, \
             tc.tile_pool(name="psJ", bufs=2, space="PSUM") as psJ:
            xT_sb = loadp.tile([P, KD, S], BF16)
            nc.scalar.dma_start(xT_sb[:], xT.rearrange("(o p) s -> p o s", p=P))
            posT_sb = loadp.tile([P, KD, S], BF16)
            nc.scalar.dma_start(posT_sb[:], posT.rearrange("(o p) s -> p o s", p=P))
            w_sbs = {}
            for nm, handle in (("Wq", Wq), ("Wk", Wk), ("Wv", Wv), ("Wp", Wp)):
                w_sb = loadp.tile([P, KD, DH2], BF16, name=f"{nm}_sb")
                nc.scalar.dma_start(w_sb[:], handle.rearrange("(o p) m -> p o m", p=P))
                w_sbs[nm] = w_sb
            vvT = loadp.tile([DH2, S], BF16)

            def proj_T(w_sb, src_sb):
                pq = psJ.tile([P, S], FP32, tag="psJ", name="pq")
                for chn in range(NCH):
                    for kt in range(KD):
                        nc.tensor.matmul(
                            pq[:, ts(chn, CH)],
                            lhsT=w_sb[:, kt, :],
                            rhs=src_sb[:, kt, ts(chn, CH)],
                            start=(kt == 0),
                            stop=(kt == KD - 1),
                        )
                return pq

            pq = proj_T(w_sbs["Wq"], xT_sb)
            nc.vector.tensor_scalar(qTu[:], pq[:], SCALE * ISQ, ueff_sb[:, 0:1], MULT, ADD)
            nc.vector.tensor_scalar(qTv[:], pq[:], SCALE * ISQ, veff_sb[:, 0:1], MULT, ADD)
            pk = proj_T(w_sbs["Wk"], xT_sb)
            nc.scalar.copy(kT[:], pk[:])
            pp_ = proj_T(w_sbs["Wp"], posT_sb)
            nc.vector.tensor_copy(pT[:], pp_[:])
            pv = proj_T(w_sbs["Wv"], xT_sb)
            nc.vector.tensor_copy(vvT[:], pv[:])
            # vv natural layout via XBAR transpose: vv[p, t, d] = vvT[d, t*128+p]
            nc.sync.dma_start_transpose(vv[:], vvT[:])
            for h in range(2):
                nc.gpsimd.tensor_copy(
                    vv_aug[h][:, :, 0:D_HEAD], vv[:, :, ds(h * D_HEAD, D_HEAD)]
                )
                nc.vector.memset(vv_aug[h][:, :, D_HEAD: D_HEAD + 1], 1.0)

        # ---- main-loop pools ----
        blk = ctx.enter_context(tc.tile_pool(name="blk", bufs=3))
        shp = ctx.enter_context(tc.tile_pool(name="shp", bufs=4))
        atp = ctx.enter_context(tc.tile_pool(name="atp", bufs=2))
        small = ctx.enter_context(tc.tile_pool(name="small", bufs=4))
        fins = ctx.enter_context(tc.tile_pool(name="fins", bufs=2))
        # one shared score pool: content tiles (freed by exp) interleave with
        # pos tiles (freed by quick DVE evacs), so content allocations never
        # wait directly on the previous exp
        psX = ctx.enter_context(tc.tile_pool(name="psX", bufs=3, space="PSUM"))
        psV = ctx.enter_context(tc.tile_pool(name="psV", bufs=2, space="PSUM"))

        # padded pos-score DRAM buffers (fp8, 64x scaled)
        PB = [dram.tile([S, S + 1], FP8, name=f"pb{h}") for h in range(2)]

        atT_tiles = {}
        sh_tiles = {}

        def pos_pair(h, jb, half, pe_t):
            """1024-col pos half -> PSUM -> one fp8 evac into pe tile (DVE)."""
            pp = psX.tile([P, HS], FP32, tag="psX", name="pp")
            for c2 in range(2):
                chn = half * 2 + c2
                nc.tensor.matmul(
                    pp[:, ts(c2, CH)],
                    lhsT=qTv[ds(h * D_HEAD, D_HEAD), ts(jb, P)],
                    rhs=pT[ds(h * D_HEAD, D_HEAD), ts(chn, CH)],
                    start=True,
                    stop=True,
                )
            nc.vector.tensor_copy(pe_t[:, 1 + half * HS: 1 + (half + 1) * HS], pp[:])

        def pos_finish(h, jb, pe_t):
            """PB write for block jb + shifted read(s) it unlocks."""
            nc.gpsimd.dma_start(PB[h][ts(jb, P), :], pe_t[:])
            # shifted read for block jb-1 depends on PB rows through jb's first row
            reads = [ib for ib in ([jb - 1, jb] if jb == NB - 1 else [jb - 1]) if ib >= 0]
            for ib in reads:
                sh = shp.tile([P, S], FP8, tag="sh", name="sh")
                flat = PB[h].flatten()
                view = flat[ds(S + ib * P * S, P * S)].rearrange("(p s) -> p s", s=S)
                nc.gpsimd.dma_start(sh[:], view)
                sh_tiles[(h, ib)] = sh

        def pos_block(h, jb):
            pe_t = blk.tile([P, S + 1], FP8, tag="pe", name="pe")
            nc.vector.memset(pe_t[:, 0:1], 0.0)
            for half in range(2):
                pos_pair(h, jb, half, pe_t)
            pos_finish(h, jb, pe_t)

        pending = []  # deferred attn@v emission closures

        def enqueue_attnv(h, half):
            atT_t = atT_tiles[(h, half)]
            for c in range(2):
                psv = psV.tile([D_HEAD + 1, CH], FP32, tag="psV", name="psv")

                def mk(kt, psv=psv, c=c):
                    def emit():
                        nc.tensor.matmul(
                            psv[:],
                            lhsT=vv_aug[h][:, kt, :],
                            rhs=atT_t[:, kt, ds(c * 4, 4), :],
                            start=(kt == 0),
                            stop=(kt == NK - 1),
                        )
                    return emit

                for kt in range(NK):
                    pending.append(mk(kt))

                def evac(psv=psv, c=c):
                    cols = ds(half * 2 * CH + c * CH, CH)
                    nc.vector.tensor_copy(
                        o_both[ds(h * D_HEAD, D_HEAD), cols],
                        psv[ds(0, D_HEAD), :],
                    )
                    nc.vector.tensor_copy(
                        sums_free[h][:, cols], psv[ds(D_HEAD, 1), :]
                    )
                pending.append(evac)

        def pump(n):
            for _ in range(n):
                if pending:
                    pending.pop(0)()

        LOOKAHEAD = 3  # pos blocks computed ahead of the consuming q block

        def iteration(h, ib):
            half = ib // HB
            if ib % HB == 0:
                atT_tiles[(h, half)] = atp.tile(
                    [P, NK, HB, P], BF16, tag="atT", name=f"atT{h}_{half}"
                )
            # attn@v for a finished half becomes available one iteration later
            if ib == HB + 1:
                enqueue_attnv(h, 0)
            if ib == 1 and h == 1:
                enqueue_attnv(0, 1)
            at = blk.tile([P, S], BF16, tag="at", name="at")
            sh = sh_tiles.pop((h, ib))
            jb = ib + LOOKAHEAD
            if jb < NB:
                pe_t = blk.tile([P, S + 1], FP8, tag="pe", name="pe")
                nc.vector.memset(pe_t[:, 0:1], 0.0)
            # content + shifted-pos accumulated per 1024 half; exp straight
            # from PSUM.  pos halves for block ib+LOOKAHEAD interleaved.
            for hf in range(2):
                pc = psX.tile([P, HS], FP32, tag="psX", name="pc")
                for c2 in range(2):
                    chn = hf * 2 + c2
                    nc.tensor.matmul(
                        pc[:, ts(c2, CH)],
                        lhsT=qTu[ds(h * D_HEAD, D_HEAD), ts(ib, P)],
                        rhs=kT[ds(h * D_HEAD, D_HEAD), ts(chn, CH)],
                        start=True,
                        stop=False,
                    )
                    nc.tensor.matmul(
                        pc[:, ts(c2, CH)],
                        lhsT=ident[:],
                        rhs=sh[:, ts(chn, CH)],
                        start=False,
                        stop=True,
                    )
                nc.scalar.activation(
                    at[:, ds(hf * HS, HS)], pc[:], Exp, scale=1.0 / SCALE,
                )
                if jb < NB:
                    pos_pair(h, jb, hf, pe_t)
                pump(3)
            if jb < NB:
                pos_finish(h, jb, pe_t)
            nc.sync.dma_start_transpose(atT_tiles[(h, half)][:, :, ib % HB, :], at[:])

        # ---- main loop ----
        for h in range(2):
            for jb in range(LOOKAHEAD):
                pos_block(h, jb)
            for ib in range(NB):
                iteration(h, ib)
        enqueue_attnv(1, 1)
        while pending:
            pump(1)

        # ---- per-head reciprocal row sums, transposed to [q-part, block] ----
        for h in range(2):
            rpad = fins.tile([16, S], BF16, tag="rpad", name="rpad")
            nc.vector.memset(rpad[:], 0.0)
            with nc.allow_low_precision("rec in bf16 for XBAR transpose"):
                nc.vector.reciprocal(rpad[0:1, :], sums_free[h][:])
            rT = fins.tile([P, 16, 16], BF16, tag="rT", name="rT")
            nc.sync.dma_start_transpose(rT[:], rpad[:])
            nc.vector.tensor_copy(recs[h][:], rT[:, :, 0])

        # ---- final projection: out[q,:] = sum_h rec_h[q] * (o_h @ Wo_h) ----
        for ib in range(NB):
            pw_a = psX.tile([P, D_MODEL], FP32, tag="psX", name="pw_a")
            nc.tensor.matmul(
                pw_a[:],
                lhsT=o_both[ds(0, D_HEAD), ts(ib, P)],
                rhs=Wo_sb[ds(0, D_HEAD), :],
                start=True, stop=True,
            )
            pw_b = psV.tile([P, D_MODEL], FP32, tag="psV", name="pw_b")
            nc.tensor.matmul(
                pw_b[:],
                lhsT=o_both[ds(D_HEAD, D_HEAD), ts(ib, P)],
                rhs=Wo_sb[ds(D_HEAD, D_HEAD), :],
                start=True, stop=True,
            )
            fa = fins.tile([P, D_MODEL], FP32, tag="fa", name="fa")
            nc.scalar.mul(fa[:], pw_a[:], recs[0][:, ib: ib + 1])
            fb = fins.tile([P, D_MODEL], FP32, tag="fb", name="fb")
            nc.scalar.mul(fb[:], pw_b[:], recs[1][:, ib: ib + 1])
            fin = fins.tile([P, D_MODEL], FP32, tag="fin", name="fin")
            nc.vector.tensor_tensor(fin[:], fa[:], fb[:], ADD)
            nc.scalar.dma_start(out_partial[ts(ib, P), :], fin[:])

    nc.finalize()
    return nc


# ---------------- host side ----------------

_NC_CACHE = {}


def _get_nc(S=S_FULL):
    if S not in _NC_CACHE:
        _NC_CACHE[S] = build_nc(S)
    return _NC_CACHE[S]


def make_in_maps(inputs, S=S_FULL, n_cores=8):
    x = np.asarray(inputs["x"], np.float32)
    pos = np.asarray(inputs["pos_embedding"], np.float32)
    Wq = np.asarray(inputs["Wq"], np.float32)
    bq = np.asarray(inputs["bq"], np.float32)
    Wk = np.asarray(inputs["Wk"], np.float32)
    Wv = np.asarray(inputs["Wv"], np.float32)
    Wp = np.asarray(inputs["Wp"], np.float32)
    u = np.asarray(inputs["u"], np.float32)
    v = np.asarray(inputs["v"], np.float32)
    Wo = np.asarray(inputs["Wo"], np.float32)

    in_maps = []
    for c in range(n_cores):
        b = c // 4
        h0 = 2 * (c % 4)
        sl = slice(h0 * D_HEAD, (h0 + 2) * D_HEAD)
        u_eff = (SCALE * ISQ * (u[h0: h0 + 2].reshape(-1) + bq[sl])).astype(np.float32)
        v_eff = (SCALE * ISQ * (v[h0: h0 + 2].reshape(-1) + bq[sl])).astype(np.float32)
        in_maps.append(
            {
                "xT": np.ascontiguousarray(x[b, :S].T).astype(BF16NP),
                "posT": np.ascontiguousarray(pos[b, :S].T).astype(BF16NP),
                "Wq": np.ascontiguousarray(Wq[:, sl]).astype(BF16NP),
                "Wk": np.ascontiguousarray(Wk[:, sl]).astype(BF16NP),
                "Wv": np.ascontiguousarray(Wv[:, sl]).astype(BF16NP),
                "Wp": np.ascontiguousarray(Wp[:, sl]).astype(BF16NP),
                "Wo": np.ascontiguousarray(Wo[sl, :]).astype(BF16NP),
                "ueff": u_eff.reshape(DH2, 1),
                "veff": v_eff.reshape(DH2, 1),
            }
        )
    return in_maps


def assemble(inputs, results, S=S_FULL):
    bv = np.asarray(inputs["bv"], np.float64)
    Wo = np.asarray(inputs["Wo"], np.float64)
    bo = np.asarray(inputs["bo"], np.float64)
    const = (bv @ Wo + bo).astype(np.float32)
    out = np.zeros((B_FULL, S, D_MODEL), np.float32)
    for c, res in enumerate(results):
        out[c // 4] += np.asarray(res["out_partial"], np.float32)
    out += const[None, None, :]
    return out


def _run(inputs, trace=False, **kw):
    nc = _get_nc(S_FULL)
    in_maps = make_in_maps(inputs, S_FULL)
    res = run_bass_kernel_spmd(nc, in_maps, list(range(8)), trace=trace, **kw)
    out = assemble(inputs, res.results, S_FULL)
    return out, res


def kernel(**inputs) -> np.ndarray:
    out, _ = _run(inputs, trace=False)
    return out


# revision 41
# speedup vs baseline: 1.0401x; 1.0401x over previous
"""Trainium2 Bass kernel for Transformer-XL style relative-position MHSA.

Problem: nn_MultiHeadSelfAttention_14989435863450
  B=2, S=2048, D=512, H=8, dh=64, fp32 I/O.

Sharding (8 cores): core c -> batch b = c//4, head pair h0 = 2*(c%4).
Each core computes its 2 heads' attention and the partial output
projection (out_slice @ Wo[slice]); host sums 4 partials per batch and
adds the constant (bv @ Wo + bo) row vector.

Math folds (exact):
  - bq folds into u,v:  u_eff = 64*(u + bq)/sqrt(D)  (64x score scaling
    for fp8 range; exp() applies scale=1/64)
  - bk adds a per-query-row constant to scores -> cancels in softmax
  - bv contributes attn-weighted 1 * bv = bv -> host-side constant
  - softmax normalization folded into the final Wo-projection evac
    (per-head reciprocal row-sum as scale)

Relative shift: pos scores are streamed to a DRAM buffer PB[S, S+1]
(fp8e4m3, 64x scaled) with rows [0 | posrow_i]; reading
PB.flat[S : S + S*S] as [S, S] is exactly Transformer-XL's
pad-reshape-slice shift (including the wrap).

v3 pipeline (from NTFF profiling of v1/v2):
  - all matmul inputs bf16; pos scores fp8 through DRAM
  - pos runs TWO q-blocks ahead; the shifted read for block ib is
    issued one iteration early (it needs PB rows through block ib+1)
  - the shifted pos tile is added into the content PSUM by an
    identity-weight matmul on the PE; exp() reads PSUM directly
    (no DVE add pass, no sc tile)
  - attention transposes via DMA XBAR on the SP ring; attn@v as
    column passes over the assembled atT
  - elementwise: DVE does pos-score fp8 evacs, ACT does exp
"""

import math
from contextlib import ExitStack

import numpy as np
import ml_dtypes

import concourse.bass as bass
import concourse.bacc as bacc_mod
import concourse.mybir as mybir
import concourse.tile as tile
from concourse.bass import ts, ds
from concourse.bass_utils import run_bass_kernel_spmd
from concourse.masks import make_identity

FP32 = mybir.dt.float32
BF16 = mybir.dt.bfloat16
FP8 = mybir.dt.float8e4

D_MODEL = 512
NUM_HEADS = 8
D_HEAD = 64
DH2 = 2 * D_HEAD
B_FULL = 2
S_FULL = 2048
P = 128
CH = 512                      # PSUM bank chunk (fp32)
ISQ = 1.0 / math.sqrt(D_MODEL)
SCALE = 64.0                  # fp8 range scaling for pos scores

Exp = mybir.ActivationFunctionType.Exp
ADD = mybir.AluOpType.add
MULT = mybir.AluOpType.mult

BF16NP = ml_dtypes.bfloat16


def build_nc(S=S_FULL):
    """Build the single-core Bass program (SPMD: same program, 8 cores)."""
    nc = bacc_mod.Bacc()
    NB = S // P               # q blocks
    NK = S // P               # key tiles
    KD = D_MODEL // P         # contraction tiles over D
    NCH = S // CH             # 512-chunks per row
    HB = NB // 2              # q blocks per half
    HS = S // 2               # columns per half

    xT = nc.declare_dram_parameter("xT", [D_MODEL, S], BF16, isOutput=False)
    posT = nc.declare_dram_parameter("posT", [D_MODEL, S], BF16, isOutput=False)
    Wq = nc.declare_dram_parameter("Wq", [D_MODEL, DH2], BF16, isOutput=False)
    Wk = nc.declare_dram_parameter("Wk", [D_MODEL, DH2], BF16, isOutput=False)
    Wv = nc.declare_dram_parameter("Wv", [D_MODEL, DH2], BF16, isOutput=False)
    Wp = nc.declare_dram_parameter("Wp", [D_MODEL, DH2], BF16, isOutput=False)
    Wo = nc.declare_dram_parameter("Wo", [DH2, D_MODEL], BF16, isOutput=False)
    ueff = nc.declare_dram_parameter("ueff", [DH2, 1], FP32, isOutput=False)
    veff = nc.declare_dram_parameter("veff", [DH2, 1], FP32, isOutput=False)
    out_partial = nc.declare_dram_parameter("out_partial", [S, D_MODEL], FP32, isOutput=True)

    with ExitStack() as ctx:
        tc = ctx.enter_context(tile.TileContext(nc))
        consts = ctx.enter_context(tc.tile_pool(name="consts", bufs=1))
        dram = ctx.enter_context(tc.tile_pool(name="dram", bufs=1, space="DRAM"))

        # ---- persistent SBUF ----
        qTu = consts.tile([DH2, S], BF16)
        qTv = consts.tile([DH2, S], BF16)
        kT = consts.tile([DH2, S], BF16)
        pT = consts.tile([DH2, S], BF16)
        vv = consts.tile([P, NK, DH2], BF16)      # natural [key, dh2]
        Wo_sb = consts.tile([DH2, D_MODEL], BF16)
        ueff_sb = consts.tile([DH2, 1], FP32)
        veff_sb = consts.tile([DH2, 1], FP32)
        ident = consts.tile([P, P], FP8)
        recs = [consts.tile([P, NB], FP32, name=f"rec{h}") for h in range(2)]
        o_both = consts.tile([DH2, S], BF16)      # unnormalized attn@v, [dh2, q]
        # vv with a ones column appended per head: attn@v's 65th output row
        # is then the softmax row-sum (no activation accumulator needed)
        vv_aug = [consts.tile([P, NK, D_HEAD + 1], BF16, name=f"vva{h}") for h in range(2)]
        sums_free = [consts.tile([1, S], FP32, name=f"sums{h}") for h in range(2)]

        make_identity(nc, ident[:])
        nc.scalar.dma_start(Wo_sb[:], Wo[:, :])
        nc.scalar.dma_start(ueff_sb[:], ueff[:, :])
        nc.scalar.dma_start(veff_sb[:], veff[:, :])

        # ---- load + projections (scoped: frees SBUF/PSUM after) ----
        with tc.tile_pool(name="loadp", bufs=1) as loadp, \
             tc.tile_pool(name="psJ", bufs=2, space="PSUM") as psJ:
            xT_sb = loadp.tile([P, KD, S], BF16)
            nc.scalar.dma_start(xT_sb[:], xT.rearrange("(o p) s -> p o s", p=P))
            posT_sb = loadp.tile([P, KD, S], BF16)
            nc.scalar.dma_start(posT_sb[:], posT.rearrange("(o p) s -> p o s", p=P))
            w_sbs = {}
            for nm, handle in (("Wq", Wq), ("Wk", Wk), ("Wv", Wv), ("Wp", Wp)):
                w_sb = loadp.tile([P, KD, DH2], BF16, name=f"{nm}_sb")
                nc.scalar.dma_start(w_sb[:], handle.rearrange("(o p) m -> p o m", p=P))
                w_sbs[nm] = w_sb
            vvT = loadp.tile([DH2, S], BF16)

            def proj_T(w_sb, src_sb):
                pq = psJ.tile([P, S], FP32, tag="psJ", name="pq")
                for chn in range(NCH):
                    for kt in range(KD):
                        nc.tensor.matmul(
                            pq[:, ts(chn, CH)],
                            lhsT=w_sb[:, kt, :],
                            rhs=src_sb[:, kt, ts(chn, CH)],
                            start=(kt == 0),
                            stop=(kt == KD - 1),
                        )
                return pq

            pq = proj_T(w_sbs["Wq"], xT_sb)
            nc.vector.tensor_scalar(qTu[:], pq[:], SCALE * ISQ, ueff_sb[:, 0:1], MULT, ADD)
            nc.vector.tensor_scalar(qTv[:], pq[:], SCALE * ISQ, veff_sb[:, 0:1], MULT, ADD)
            pk = proj_T(w_sbs["Wk"], xT_sb)
            nc.scalar.copy(kT[:], pk[:])
            pp_ = proj_T(w_sbs["Wp"], posT_sb)
            nc.vector.tensor_copy(pT[:], pp_[:])
            pv = proj_T(w_sbs["Wv"], xT_sb)
            nc.vector.tensor_copy(vvT[:], pv[:])
            # vv natural layout via XBAR transpose: vv[p, t, d] = vvT[d, t*128+p]
            nc.sync.dma_start_transpose(vv[:], vvT[:])
            for h in range(2):
                nc.gpsimd.tensor_copy(
                    vv_aug[h][:, :, 0:D_HEAD], vv[:, :, ds(h * D_HEAD, D_HEAD)]
                )
                nc.vector.memset(vv_aug[h][:, :, D_HEAD: D_HEAD + 1], 1.0)

        # ---- main-loop pools ----
        blk = ctx.enter_context(tc.tile_pool(name="blk", bufs=3))
        shp = ctx.enter_context(tc.tile_pool(name="shp", bufs=4))
        atp = ctx.enter_context(tc.tile_pool(name="atp", bufs=2))
        small = ctx.enter_context(tc.tile_pool(name="small", bufs=4))
        fins = ctx.enter_context(tc.tile_pool(name="fins", bufs=2))
        psC = ctx.enter_context(tc.tile_pool(name="psC", bufs=2, space="PSUM"))
        psP = ctx.enter_context(tc.tile_pool(name="psP", bufs=2, space="PSUM"))
        psV = ctx.enter_context(tc.tile_pool(name="psV", bufs=2, space="PSUM"))

        # padded pos-score DRAM buffers (fp8, 64x scaled)
        PB = [dram.tile([S, S + 1], FP8, name=f"pb{h}") for h in range(2)]

        atT_tiles = {}
        sh_tiles = {}

        def pos_chunk(h, jb, chn, pe_t):
            """one 512-col pos chunk -> PSUM -> fp8 evac into pe tile (DVE)."""
            pp = psP.tile([P, CH], FP32, tag="psP", name="pp")
            nc.tensor.matmul(
                pp[:],
                lhsT=qTv[ds(h * D_HEAD, D_HEAD), ts(jb, P)],
                rhs=pT[ds(h * D_HEAD, D_HEAD), ts(chn, CH)],
                start=True,
                stop=True,
            )
            nc.vector.tensor_copy(pe_t[:, 1 + chn * CH: 1 + (chn + 1) * CH], pp[:])

        def pos_finish(h, jb, pe_t):
            """PB write for block jb + shifted read(s) it unlocks (SWDGE:
            keeps the SP/ACT queues clear for XBAR and exp)."""
            nc.gpsimd.dma_start(PB[h][ts(jb, P), :], pe_t[:])
            # shifted read for block jb-1 depends on PB rows through jb's first row
            reads = [ib for ib in ([jb - 1, jb] if jb == NB - 1 else [jb - 1]) if ib >= 0]
            for ib in reads:
                sh = shp.tile([P, S], FP8, tag="sh", name="sh")
                flat = PB[h].flatten()
                view = flat[ds(S + ib * P * S, P * S)].rearrange("(p s) -> p s", s=S)
                nc.gpsimd.dma_start(sh[:], view)
                sh_tiles[(h, ib)] = sh

        def pos_block(h, jb):
            pe_t = blk.tile([P, S + 1], FP8, tag="pe", name="pe")
            nc.vector.memset(pe_t[:, 0:1], 0.0)
            for chn in range(NCH):
                pos_chunk(h, jb, chn, pe_t)
            pos_finish(h, jb, pe_t)

        pending = []  # deferred attn@v emission closures

        def enqueue_attnv(h, half):
            atT_t = atT_tiles[(h, half)]
            for c in range(2):
                psv = psV.tile([D_HEAD + 1, CH], FP32, tag="psV", name="psv")

                def mk(kt, psv=psv, c=c):
                    def emit():
                        nc.tensor.matmul(
                            psv[:],
                            lhsT=vv_aug[h][:, kt, :],
                            rhs=atT_t[:, kt, ds(c * 4, 4), :],
                            start=(kt == 0),
                            stop=(kt == NK - 1),
                        )
                    return emit

                for kt in range(NK):
                    pending.append(mk(kt))

                def evac(psv=psv, c=c):
                    cols = ds(half * 2 * CH + c * CH, CH)
                    nc.vector.tensor_copy(
                        o_both[ds(h * D_HEAD, D_HEAD), cols],
                        psv[ds(0, D_HEAD), :],
                    )
                    nc.vector.tensor_copy(
                        sums_free[h][:, cols], psv[ds(D_HEAD, 1), :]
                    )
                pending.append(evac)

        def pump(n):
            for _ in range(n):
                if pending:
                    pending.pop(0)()

        LOOKAHEAD = 3  # pos blocks computed ahead of the consuming q block

        def iteration(h, ib):
            half = ib // HB
            if ib % HB == 0:
                atT_tiles[(h, half)] = atp.tile(
                    [P, NK, HB, P], BF16, tag="atT", name=f"atT{h}_{half}"
                )
            # attn@v for a finished half becomes available one iteration later
            if ib == HB + 1:
                enqueue_attnv(h, 0)
            if ib == 1 and h == 1:
                enqueue_attnv(0, 1)
            at = blk.tile([P, S], BF16, tag="at", name="at")
            sh = sh_tiles.pop((h, ib))
            jb = ib + LOOKAHEAD
            if jb < NB:
                pe_t = blk.tile([P, S + 1], FP8, tag="pe", name="pe")
                nc.vector.memset(pe_t[:, 0:1], 0.0)
            # content + shifted-pos accumulated per half; exp straight from PSUM
            for hf in range(2):
                pc = psC.tile([P, HS], FP32, tag="pc", name="pc")
                for c2 in range(2):
                    chn = hf * 2 + c2
                    nc.tensor.matmul(
                        pc[:, ts(c2, CH)],
                        lhsT=qTu[ds(h * D_HEAD, D_HEAD), ts(ib, P)],
                        rhs=kT[ds(h * D_HEAD, D_HEAD), ts(chn, CH)],
                        start=True,
                        stop=False,
                    )
                    nc.tensor.matmul(
                        pc[:, ts(c2, CH)],
                        lhsT=ident[:],
                        rhs=sh[:, ts(chn, CH)],
                        start=False,
                        stop=True,
                    )
                nc.scalar.activation(
                    at[:, ds(hf * HS, HS)], pc[:], Exp, scale=1.0 / SCALE,
                )
                # pos for block ib+LOOKAHEAD interleaved between the halves
                if jb < NB:
                    pos_chunk(h, jb, 2 * hf + 0, pe_t)
                    pos_chunk(h, jb, 2 * hf + 1, pe_t)
                    if hf == 1:
                        pos_finish(h, jb, pe_t)
            nc.sync.dma_start_transpose(atT_tiles[(h, half)][:, :, ib % HB, :], at[:])
            pump(6)

        # ---- main loop ----
        for h in range(2):
            for jb in range(LOOKAHEAD):
                pos_block(h, jb)
            for ib in range(NB):
                iteration(h, ib)
        enqueue_attnv(1, 1)
        while pending:
            pump(1)

        # ---- per-head reciprocal row sums, transposed to [q-part, block] ----
        for h in range(2):
            rpad = fins.tile([16, S], BF16, tag="rpad", name="rpad")
            nc.vector.memset(rpad[:], 0.0)
            with nc.allow_low_precision("rec in bf16 for XBAR transpose"):
                nc.vector.reciprocal(rpad[0:1, :], sums_free[h][:])
            rT = fins.tile([P, 16, 16], BF16, tag="rT", name="rT")
            nc.sync.dma_start_transpose(rT[:], rpad[:])
            nc.vector.tensor_copy(recs[h][:], rT[:, :, 0])

        # ---- final projection: out[q,:] = sum_h rec_h[q] * (o_h @ Wo_h) ----
        for ib in range(NB):
            pw_a = psP.tile([P, D_MODEL], FP32, tag="psP", name="pw_a")
            nc.tensor.matmul(
                pw_a[:],
                lhsT=o_both[ds(0, D_HEAD), ts(ib, P)],
                rhs=Wo_sb[ds(0, D_HEAD), :],
                start=True, stop=True,
            )
            pw_b = psV.tile([P, D_MODEL], FP32, tag="psV", name="pw_b")
            nc.tensor.matmul(
                pw_b[:],
                lhsT=o_both[ds(D_HEAD, D_HEAD), ts(ib, P)],
                rhs=Wo_sb[ds(D_HEAD, D_HEAD), :],
                start=True, stop=True,
            )
            fa = fins.tile([P, D_MODEL], FP32, tag="fa", name="fa")
            nc.scalar.mul(fa[:], pw_a[:], recs[0][:, ib: ib + 1])
            fb = fins.tile([P, D_MODEL], FP32, tag="fb", name="fb")
            nc.scalar.mul(fb[:], pw_b[:], recs[1][:, ib: ib + 1])
            fin = fins.tile([P, D_MODEL], FP32, tag="fin", name="fin")
            nc.vector.tensor_tensor(fin[:], fa[:], fb[:], ADD)
            nc.scalar.dma_start(out_partial[ts(ib, P), :], fin[:])

    nc.finalize()
    return nc


# ---------------- host side ----------------

_NC_CACHE = {}


def _get_nc(S=S_FULL):
    if S not in _NC_CACHE:
        _NC_CACHE[S] = build_nc(S)
    return _NC_CACHE[S]


def make_in_maps(inputs, S=S_FULL, n_cores=8):
    x = np.asarray(inputs["x"], np.float32)
    pos = np.asarray(inputs["pos_embedding"], np.float32)
    Wq = np.asarray(inputs["Wq"], np.float32)
    bq = np.asarray(inputs["bq"], np.float32)
    Wk = np.asarray(inputs["Wk"], np.float32)
    Wv = np.asarray(inputs["Wv"], np.float32)
    Wp = np.asarray(inputs["Wp"], np.float32)
    u = np.asarray(inputs["u"], np.float32)
    v = np.asarray(inputs["v"], np.float32)
    Wo = np.asarray(inputs["Wo"], np.float32)

    in_maps = []
    for c in range(n_cores):
        b = c // 4
        h0 = 2 * (c % 4)
        sl = slice(h0 * D_HEAD, (h0 + 2) * D_HEAD)
        u_eff = (SCALE * ISQ * (u[h0: h0 + 2].reshape(-1) + bq[sl])).astype(np.float32)
        v_eff = (SCALE * ISQ * (v[h0: h0 + 2].reshape(-1) + bq[sl])).astype(np.float32)
        in_maps.append(
            {
                "xT": np.ascontiguousarray(x[b, :S].T).astype(BF16NP),
                "posT": np.ascontiguousarray(pos[b, :S].T).astype(BF16NP),
                "Wq": np.ascontiguousarray(Wq[:, sl]).astype(BF16NP),
                "Wk": np.ascontiguousarray(Wk[:, sl]).astype(BF16NP),
                "Wv": np.ascontiguousarray(Wv[:, sl]).astype(BF16NP),
                "Wp": np.ascontiguousarray(Wp[:, sl]).astype(BF16NP),
                "Wo": np.ascontiguousarray(Wo[sl, :]).astype(BF16NP),
                "ueff": u_eff.reshape(DH2, 1),
                "veff": v_eff.reshape(DH2, 1),
            }
        )
    return in_maps


def assemble(inputs, results, S=S_FULL):
    bv = np.asarray(inputs["bv"], np.float64)
    Wo = np.asarray(inputs["Wo"], np.float64)
    bo = np.asarray(inputs["bo"], np.float64)
    const = (bv @ Wo + bo).astype(np.float32)
    out = np.zeros((B_FULL, S, D_MODEL), np.float32)
    for c, res in enumerate(results):
        out[c // 4] += np.asarray(res["out_partial"], np.float32)
    out += const[None, None, :]
    return out


def _run(inputs, trace=False, **kw):
    nc = _get_nc(S_FULL)
    in_maps = make_in_maps(inputs, S_FULL)
    res = run_bass_kernel_spmd(nc, in_maps, list(range(8)), trace=trace, **kw)
    out = assemble(inputs, res.results, S_FULL)
    return out, res


def kernel(**inputs) -> np.ndarray:
    out, _ = _run(inputs, trace=False)
    return out


# revision 42
# speedup vs baseline: 1.1820x; 1.1365x over previous
"""Trainium2 Bass kernel for Transformer-XL style relative-position MHSA.

Problem: nn_MultiHeadSelfAttention_14989435863450
  B=2, S=2048, D=512, H=8, dh=64, fp32 I/O.

Sharding (8 cores): core c -> batch b = c//4, head pair h0 = 2*(c%4).
Each core computes its 2 heads' attention and the partial output
projection (out_slice @ Wo[slice]); host sums 4 partials per batch and
adds the constant (bv @ Wo + bo) row vector.

Math folds (exact):
  - bq folds into u,v:  u_eff = 64*(u + bq)/sqrt(D)  (64x score scaling
    for fp8 range; exp() applies scale=1/64)
  - bk adds a per-query-row constant to scores -> cancels in softmax
  - bv contributes attn-weighted 1 * bv = bv -> host-side constant
  - softmax normalization folded into the final Wo-projection evac
    (per-head reciprocal row-sum as scale)

Relative shift: pos scores are streamed to a DRAM buffer PB[S, S+1]
(fp8e4m3, 64x scaled) with rows [0 | posrow_i]; reading
PB.flat[S : S + S*S] as [S, S] is exactly Transformer-XL's
pad-reshape-slice shift (including the wrap).

v3 pipeline (from NTFF profiling of v1/v2):
  - all matmul inputs bf16; pos scores fp8 through DRAM
  - pos runs TWO q-blocks ahead; the shifted read for block ib is
    issued one iteration early (it needs PB rows through block ib+1)
  - the shifted pos tile is added into the content PSUM by an
    identity-weight matmul on the PE; exp() reads PSUM directly
    (no DVE add pass, no sc tile)
  - attention transposes via DMA XBAR on the SP ring; attn@v as
    column passes over the assembled atT
  - elementwise: DVE does pos-score fp8 evacs, ACT does exp
"""

import math
from contextlib import ExitStack

import numpy as np
import ml_dtypes

import concourse.bass as bass
import concourse.bacc as bacc_mod
import concourse.mybir as mybir
import concourse.tile as tile
from concourse.bass import ts, ds
from concourse.bass_utils import run_bass_kernel_spmd
from concourse.masks import make_identity

FP32 = mybir.dt.float32
BF16 = mybir.dt.bfloat16
FP8 = mybir.dt.float8e4

D_MODEL = 512
NUM_HEADS = 8
D_HEAD = 64
DH2 = 2 * D_HEAD
B_FULL = 2
S_FULL = 2048
P = 128
CH = 512                      # PSUM bank chunk (fp32)
ISQ = 1.0 / math.sqrt(D_MODEL)
SCALE = 64.0                  # fp8 range scaling for pos scores

Exp = mybir.ActivationFunctionType.Exp
ADD = mybir.AluOpType.add
MULT = mybir.AluOpType.mult

BF16NP = ml_dtypes.bfloat16


def build_nc(S=S_FULL):
    """Build the single-core Bass program (SPMD: same program, 8 cores)."""
    nc = bacc_mod.Bacc()
    NB = S // P               # q blocks
    NK = S // P               # key tiles
    KD = D_MODEL // P         # contraction tiles over D
    NCH = S // CH             # 512-chunks per row
    HB = NB // 2              # q blocks per half
    HS = S // 2               # columns per half

    xT = nc.declare_dram_parameter("xT", [D_MODEL, S], BF16, isOutput=False)
    posT = nc.declare_dram_parameter("posT", [D_MODEL, S], BF16, isOutput=False)
    Wq = nc.declare_dram_parameter("Wq", [D_MODEL, DH2], BF16, isOutput=False)
    Wk = nc.declare_dram_parameter("Wk", [D_MODEL, DH2], BF16, isOutput=False)
    Wv = nc.declare_dram_parameter("Wv", [D_MODEL, DH2], BF16, isOutput=False)
    Wp = nc.declare_dram_parameter("Wp", [D_MODEL, DH2], BF16, isOutput=False)
    Wo = nc.declare_dram_parameter("Wo", [DH2, D_MODEL], BF16, isOutput=False)
    ueff = nc.declare_dram_parameter("ueff", [DH2, 1], FP32, isOutput=False)
    veff = nc.declare_dram_parameter("veff", [DH2, 1], FP32, isOutput=False)
    out_partial = nc.declare_dram_parameter("out_partial", [S, D_MODEL], FP32, isOutput=True)

    with ExitStack() as ctx:
        tc = ctx.enter_context(tile.TileContext(nc))
        consts = ctx.enter_context(tc.tile_pool(name="consts", bufs=1))
        dram = ctx.enter_context(tc.tile_pool(name="dram", bufs=1, space="DRAM"))

        # ---- persistent SBUF ----
        qTu = consts.tile([DH2, S], BF16)
        qTv = consts.tile([DH2, S], BF16)
        kT = consts.tile([DH2, S], BF16)
        pT = consts.tile([DH2, S], BF16)
        vv = consts.tile([P, NK, DH2], BF16)      # natural [key, dh2]
        Wo_sb = consts.tile([DH2, D_MODEL], BF16)
        ueff_sb = consts.tile([DH2, 1], FP32)
        veff_sb = consts.tile([DH2, 1], FP32)
        ident = consts.tile([P, P], FP8)
        recs = [consts.tile([P, NB], FP32, name=f"rec{h}") for h in range(2)]
        o_both = consts.tile([DH2, S], BF16)      # unnormalized attn@v, [dh2, q]
        # vv with a ones column appended per head: attn@v's 65th output row
        # is then the softmax row-sum (no activation accumulator needed)
        vv_aug = [consts.tile([P, NK, D_HEAD + 1], BF16, name=f"vva{h}") for h in range(2)]
        sums_free = [consts.tile([1, S], FP32, name=f"sums{h}") for h in range(2)]

        make_identity(nc, ident[:])
        nc.scalar.dma_start(Wo_sb[:], Wo[:, :])
        nc.scalar.dma_start(ueff_sb[:], ueff[:, :])
        nc.scalar.dma_start(veff_sb[:], veff[:, :])

        # ---- load + projections (scoped: frees SBUF/PSUM after) ----
        with tc.tile_pool(name="loadp", bufs=1) as loadp, \
             tc.tile_pool(name="psJ", bufs=2, space="PSUM") as psJ:
            xT_sb = loadp.tile([P, KD, S], BF16)
            nc.scalar.dma_start(xT_sb[:], xT.rearrange("(o p) s -> p o s", p=P))
            posT_sb = loadp.tile([P, KD, S], BF16)
            nc.scalar.dma_start(posT_sb[:], posT.rearrange("(o p) s -> p o s", p=P))
            w_sbs = {}
            for nm, handle in (("Wq", Wq), ("Wk", Wk), ("Wv", Wv), ("Wp", Wp)):
                w_sb = loadp.tile([P, KD, DH2], BF16, name=f"{nm}_sb")
                nc.scalar.dma_start(w_sb[:], handle.rearrange("(o p) m -> p o m", p=P))
                w_sbs[nm] = w_sb
            vvT = loadp.tile([DH2, S], BF16)

            def proj_T(w_sb, src_sb):
                pq = psJ.tile([P, S], FP32, tag="psJ", name="pq")
                for chn in range(NCH):
                    for kt in range(KD):
                        nc.tensor.matmul(
                            pq[:, ts(chn, CH)],
                            lhsT=w_sb[:, kt, :],
                            rhs=src_sb[:, kt, ts(chn, CH)],
                            start=(kt == 0),
                            stop=(kt == KD - 1),
                        )
                return pq

            pq = proj_T(w_sbs["Wq"], xT_sb)
            nc.vector.tensor_scalar(qTu[:], pq[:], SCALE * ISQ, ueff_sb[:, 0:1], MULT, ADD)
            nc.vector.tensor_scalar(qTv[:], pq[:], SCALE * ISQ, veff_sb[:, 0:1], MULT, ADD)
            pk = proj_T(w_sbs["Wk"], xT_sb)
            nc.scalar.copy(kT[:], pk[:])
            pp_ = proj_T(w_sbs["Wp"], posT_sb)
            nc.vector.tensor_copy(pT[:], pp_[:])
            pv = proj_T(w_sbs["Wv"], xT_sb)
            nc.vector.tensor_copy(vvT[:], pv[:])
            # vv natural layout via XBAR transpose: vv[p, t, d] = vvT[d, t*128+p]
            nc.sync.dma_start_transpose(vv[:], vvT[:])
            for h in range(2):
                nc.gpsimd.tensor_copy(
                    vv_aug[h][:, :, 0:D_HEAD], vv[:, :, ds(h * D_HEAD, D_HEAD)]
                )
                nc.vector.memset(vv_aug[h][:, :, D_HEAD: D_HEAD + 1], 1.0)

        # ---- main-loop pools ----
        blk = ctx.enter_context(tc.tile_pool(name="blk", bufs=3))
        shp = ctx.enter_context(tc.tile_pool(name="shp", bufs=4))
        atp = ctx.enter_context(tc.tile_pool(name="atp", bufs=2))
        small = ctx.enter_context(tc.tile_pool(name="small", bufs=4))
        fins = ctx.enter_context(tc.tile_pool(name="fins", bufs=2))
        psC = ctx.enter_context(tc.tile_pool(name="psC", bufs=2, space="PSUM"))
        psP = ctx.enter_context(tc.tile_pool(name="psP", bufs=2, space="PSUM"))
        psV = ctx.enter_context(tc.tile_pool(name="psV", bufs=2, space="PSUM"))

        # padded pos-score DRAM buffers (fp8, 64x scaled)
        PB = [dram.tile([S, S + 1], FP8, name=f"pb{h}") for h in range(2)]

        atT_tiles = {}
        sh_tiles = {}

        def pos_chunk(h, jb, chn, pe_t):
            """one 512-col pos chunk -> PSUM -> fp8 evac into pe tile (DVE)."""
            pp = psP.tile([P, CH], FP32, tag="psP", name="pp")
            nc.tensor.matmul(
                pp[:],
                lhsT=qTv[ds(h * D_HEAD, D_HEAD), ts(jb, P)],
                rhs=pT[ds(h * D_HEAD, D_HEAD), ts(chn, CH)],
                start=True,
                stop=True,
            )
            nc.vector.tensor_copy(pe_t[:, 1 + chn * CH: 1 + (chn + 1) * CH], pp[:])

        def pos_finish(h, jb, pe_t):
            """PB write for block jb + shifted read(s) it unlocks."""
            nc.sync.dma_start(PB[h][ts(jb, P), :], pe_t[:])
            # shifted read for block jb-1 depends on PB rows through jb's first row
            reads = [ib for ib in ([jb - 1, jb] if jb == NB - 1 else [jb - 1]) if ib >= 0]
            for ib in reads:
                sh = shp.tile([P, S], FP8, tag="sh", name="sh")
                flat = PB[h].flatten()
                view = flat[ds(S + ib * P * S, P * S)].rearrange("(p s) -> p s", s=S)
                nc.scalar.dma_start(sh[:], view)
                sh_tiles[(h, ib)] = sh

        def pos_block(h, jb):
            pe_t = blk.tile([P, S + 1], FP8, tag="pe", name="pe")
            nc.vector.memset(pe_t[:, 0:1], 0.0)
            for chn in range(NCH):
                pos_chunk(h, jb, chn, pe_t)
            pos_finish(h, jb, pe_t)

        pending = []  # deferred attn@v emission closures

        def enqueue_attnv(h, half):
            atT_t = atT_tiles[(h, half)]
            for c in range(2):
                psv = psV.tile([D_HEAD + 1, CH], FP32, tag="psV", name="psv")

                def mk(kt, psv=psv, c=c):
                    def emit():
                        nc.tensor.matmul(
                            psv[:],
                            lhsT=vv_aug[h][:, kt, :],
                            rhs=atT_t[:, kt, ds(c * 4, 4), :],
                            start=(kt == 0),
                            stop=(kt == NK - 1),
                        )
                    return emit

                for kt in range(NK):
                    pending.append(mk(kt))

                def evac(psv=psv, c=c):
                    cols = ds(half * 2 * CH + c * CH, CH)
                    nc.vector.tensor_copy(
                        o_both[ds(h * D_HEAD, D_HEAD), cols],
                        psv[ds(0, D_HEAD), :],
                    )
                    nc.vector.tensor_copy(
                        sums_free[h][:, cols], psv[ds(D_HEAD, 1), :]
                    )
                pending.append(evac)

        def pump(n):
            for _ in range(n):
                if pending:
                    pending.pop(0)()

        LOOKAHEAD = 3  # pos blocks computed ahead of the consuming q block

        def iteration(h, ib):
            half = ib // HB
            if ib % HB == 0:
                atT_tiles[(h, half)] = atp.tile(
                    [P, NK, HB, P], BF16, tag="atT", name=f"atT{h}_{half}"
                )
            # attn@v for a finished half becomes available one iteration later
            if ib == HB + 1:
                enqueue_attnv(h, 0)
            if ib == 1 and h == 1:
                enqueue_attnv(0, 1)
            at = blk.tile([P, S], BF16, tag="at", name="at")
            sh = sh_tiles.pop((h, ib))
            jb = ib + LOOKAHEAD
            if jb < NB:
                pe_t = blk.tile([P, S + 1], FP8, tag="pe", name="pe")
                nc.vector.memset(pe_t[:, 0:1], 0.0)
            # content + shifted-pos accumulated per half; exp straight from PSUM
            for hf in range(2):
                pc = psC.tile([P, HS], FP32, tag="pc", name="pc")
                for c2 in range(2):
                    chn = hf * 2 + c2
                    nc.tensor.matmul(
                        pc[:, ts(c2, CH)],
                        lhsT=qTu[ds(h * D_HEAD, D_HEAD), ts(ib, P)],
                        rhs=kT[ds(h * D_HEAD, D_HEAD), ts(chn, CH)],
                        start=True,
                        stop=False,
                    )
                    nc.tensor.matmul(
                        pc[:, ts(c2, CH)],
                        lhsT=ident[:],
                        rhs=sh[:, ts(chn, CH)],
                        start=False,
                        stop=True,
                    )
                nc.scalar.activation(
                    at[:, ds(hf * HS, HS)], pc[:], Exp, scale=1.0 / SCALE,
                )
                # pos for block ib+LOOKAHEAD interleaved between the halves
                if jb < NB:
                    pos_chunk(h, jb, 2 * hf + 0, pe_t)
                    pos_chunk(h, jb, 2 * hf + 1, pe_t)
                    if hf == 1:
                        pos_finish(h, jb, pe_t)
            nc.sync.dma_start_transpose(atT_tiles[(h, half)][:, :, ib % HB, :], at[:])
            pump(6)

        # ---- main loop ----
        for h in range(2):
            for jb in range(LOOKAHEAD):
                pos_block(h, jb)
            for ib in range(NB):
                iteration(h, ib)
        enqueue_attnv(1, 1)
        while pending:
            pump(1)

        # ---- per-head reciprocal row sums, transposed to [q-part, block] ----
        for h in range(2):
            rpad = fins.tile([16, S], BF16, tag="rpad", name="rpad")
            nc.vector.memset(rpad[:], 0.0)
            with nc.allow_low_precision("rec in bf16 for XBAR transpose"):
                nc.vector.reciprocal(rpad[0:1, :], sums_free[h][:])
            rT = fins.tile([P, 16, 16], BF16, tag="rT", name="rT")
            nc.sync.dma_start_transpose(rT[:], rpad[:])
            nc.vector.tensor_copy(recs[h][:], rT[:, :, 0])

        # ---- final projection: out[q,:] = sum_h rec_h[q] * (o_h @ Wo_h) ----
        for ib in range(NB):
            pw_a = psP.tile([P, D_MODEL], FP32, tag="psP", name="pw_a")
            nc.tensor.matmul(
                pw_a[:],
                lhsT=o_both[ds(0, D_HEAD), ts(ib, P)],
                rhs=Wo_sb[ds(0, D_HEAD), :],
                start=True, stop=True,
            )
            pw_b = psV.tile([P, D_MODEL], FP32, tag="psV", name="pw_b")
            nc.tensor.matmul(
                pw_b[:],
                lhsT=o_both[ds(D_HEAD, D_HEAD), ts(ib, P)],
                rhs=Wo_sb[ds(D_HEAD, D_HEAD), :],
                start=True, stop=True,
            )
            fa = fins.tile([P, D_MODEL], FP32, tag="fa", name="fa")
            nc.scalar.mul(fa[:], pw_a[:], recs[0][:, ib: ib + 1])
            fb = fins.tile([P, D_MODEL], FP32, tag="fb", name="fb")
            nc.scalar.mul(fb[:], pw_b[:], recs[1][:, ib: ib + 1])
            fin = fins.tile([P, D_MODEL], FP32, tag="fin", name="fin")
            nc.vector.tensor_tensor(fin[:], fa[:], fb[:], ADD)
            nc.scalar.dma_start(out_partial[ts(ib, P), :], fin[:])

    nc.finalize()
    return nc


# ---------------- host side ----------------

_NC_CACHE = {}


def _get_nc(S=S_FULL):
    if S not in _NC_CACHE:
        _NC_CACHE[S] = build_nc(S)
    return _NC_CACHE[S]


def make_in_maps(inputs, S=S_FULL, n_cores=8):
    x = np.asarray(inputs["x"], np.float32)
    pos = np.asarray(inputs["pos_embedding"], np.float32)
    Wq = np.asarray(inputs["Wq"], np.float32)
    bq = np.asarray(inputs["bq"], np.float32)
    Wk = np.asarray(inputs["Wk"], np.float32)
    Wv = np.asarray(inputs["Wv"], np.float32)
    Wp = np.asarray(inputs["Wp"], np.float32)
    u = np.asarray(inputs["u"], np.float32)
    v = np.asarray(inputs["v"], np.float32)
    Wo = np.asarray(inputs["Wo"], np.float32)

    in_maps = []
    for c in range(n_cores):
        b = c // 4
        h0 = 2 * (c % 4)
        sl = slice(h0 * D_HEAD, (h0 + 2) * D_HEAD)
        u_eff = (SCALE * ISQ * (u[h0: h0 + 2].reshape(-1) + bq[sl])).astype(np.float32)
        v_eff = (SCALE * ISQ * (v[h0: h0 + 2].reshape(-1) + bq[sl])).astype(np.float32)
        in_maps.append(
            {
                "xT": np.ascontiguousarray(x[b, :S].T).astype(BF16NP),
                "posT": np.ascontiguousarray(pos[b, :S].T).astype(BF16NP),
                "Wq": np.ascontiguousarray(Wq[:, sl]).astype(BF16NP),
                "Wk": np.ascontiguousarray(Wk[:, sl]).astype(BF16NP),
                "Wv": np.ascontiguousarray(Wv[:, sl]).astype(BF16NP),
                "Wp": np.ascontiguousarray(Wp[:, sl]).astype(BF16NP),
                "Wo": np.ascontiguousarray(Wo[sl, :]).astype(BF16NP),
                "ueff": u_eff.reshape(DH2, 1),
                "veff": v_eff.reshape(DH2, 1),
            }
        )
    return in_maps


def assemble(inputs, results, S=S_FULL):
    bv = np.asarray(inputs["bv"], np.float64)
    Wo = np.asarray(inputs["Wo"], np.float64)
    bo = np.asarray(inputs["bo"], np.float64)
    const = (bv @ Wo + bo).astype(np.float32)
    out = np.zeros((B_FULL, S, D_MODEL), np.float32)
    for c, res in enumerate(results):
        out[c // 4] += np.asarray(res["out_partial"], np.float32)
    out += const[None, None, :]
    return out


def _run(inputs, trace=False, **kw):
    nc = _get_nc(S_FULL)
    in_maps = make_in_maps(inputs, S_FULL)
    res = run_bass_kernel_spmd(nc, in_maps, list(range(8)), trace=trace, **kw)
    out = assemble(inputs, res.results, S_FULL)
    return out, res


def kernel(**inputs) -> np.ndarray:
    out, _ = _run(inputs, trace=False)
    return out
